# revision 6
# baseline (speedup 1.0000x reference)
import os

import numpy as np

import concourse.bacc as bacc
import concourse.bass as bass
import concourse.mybir as mybir
import concourse.tile as tile
from concourse.bass import ts
from concourse.bass_utils import run_bass_kernel_spmd

NUM_OPTIONS = 8
NUM_INPUTS = 128
STATE_HIDDEN = 1024
HIDDEN = 128
NUM_ACTIONS = 32
LOG_STD_MIN = -20.0
LOG_STD_MAX = 2.0

MM_DT = getattr(mybir.dt, os.environ.get("KERNEL_MM_DT", "float16"))
OUT_DT = mybir.dt.float16
WARMUP_MMS = int(os.environ.get("KERNEL_WARMUP", "12"))

_kernel_cache: dict = {}


def _groups(cap: int) -> list[tuple[int, int, int]]:
    out = []
    s = 0
    while s < cap:
        n = min(cap - s, 576)
        c0 = min(n, 512)
        out.append((s, c0, n - c0))
        s += n
    return out


def _build(cap: int, mm_dt) -> bass.Bass:
    f32 = mybir.dt.float32
    n_h1 = STATE_HIDDEN // 128
    nc = bacc.Bacc(trn_type="TRN2", debug=False)

    awid = cap + 2 * STATE_HIDDEN + 2 * NUM_ACTIONS
    a = nc.dram_tensor("a", [128, awid], mm_dt, kind="ExternalInput").ap()
    outT = nc.dram_tensor("outT", [2 * NUM_ACTIONS, cap], OUT_DT, kind="ExternalOutput").ap()

    with tile.TileContext(nc) as tc:
        with (
            tc.tile_pool(name="ins", bufs=1) as ipool,
            tc.tile_pool(name="acts", bufs=1) as apool,
            tc.tile_pool(name="outs", bufs=1) as opool,
            tc.tile_pool(name="ps1", bufs=3, space="PSUM") as ps1,
            tc.tile_pool(name="psb", bufs=1, space="PSUM") as psb,
            tc.tile_pool(name="ps2", bufs=1, space="PSUM") as ps2,
            tc.tile_pool(name="ps3", bufs=1, space="PSUM") as ps3,
            tc.tile_pool(name="psA", bufs=1, space="PSUM") as psA,
            tc.tile_pool(name="psB", bufs=1, space="PSUM") as psB,
        ):
            s1 = min(cap + 256, awid)
            s2 = min(s1 + 512, awid)
            s3 = min(s2 + 512, awid)
            asb = ipool.tile([128, awid], mm_dt)
            nc.sync.dma_start(out=asb[:, :s1], in_=a[:, :s1])
            if s1 < awid:
                nc.sync.dma_start(out=asb[:, s1:s2], in_=a[:, s1:s2])
            if s2 < awid:
                nc.sync.dma_start(out=asb[:, s2:s3], in_=a[:, s2:s3])
            if s3 < awid:
                nc.sync.dma_start(out=asb[:, s3:], in_=a[:, s3:])

            bf16 = mybir.dt.bfloat16
            wz = ipool.tile([128, 256], bf16)
            nc.gpsimd.memset(wz, 0)
            pw = ps2.tile([128, 256], f32, tag="p2")
            for _ in range(WARMUP_MMS):
                nc.tensor.matmul(pw, wz[:, :128], wz, start=True, stop=True)
            for _ in range(4):
                nc.tensor.matmul(
                    pw[:, :64], wz[:, :128], wz[:, :64], start=True, stop=True
                )

            xs = asb[:, :cap]
            w1s = asb[:, cap : cap + STATE_HIDDEN]
            w2s = asb[:, cap + STATE_HIDDEN : cap + 2 * STATE_HIDDEN]
            whs = asb[:, cap + 2 * STATE_HIDDEN :]

            relu_engines = (
                lambda o, i: nc.scalar.activation(o, i, mybir.ActivationFunctionType.Relu),
                lambda o, i: nc.vector.tensor_scalar_max(o, i, 0.0),
            )

            for g0, c0, c1 in _groups(cap):
                x0 = xs[:, g0 : g0 + c0]
                x1 = xs[:, g0 + c0 : g0 + c0 + c1]
                h1a = apool.tile([128, n_h1, c0], mm_dt, tag="h1a")
                if c1:
                    h1b = apool.tile([128, n_h1 * c1], mm_dt, tag="h1b")
                    p1b = psb.tile([128, n_h1 * c1], f32, tag="p1b")
                for j in range(n_h1):
                    p1 = ps1.tile([128, c0], f32, tag="p1")
                    nc.tensor.matmul(p1, w1s[:, ts(j, 128)], x0, start=True, stop=True)
                    if c1:
                        nc.tensor.matmul(
                            p1b[:, j * c1 : (j + 1) * c1], w1s[:, ts(j, 128)], x1,
                            start=True, stop=True,
                        )
                    relu_engines[j % 2](h1a[:, j, :], p1)
                if c1:
                    relu_engines[n_h1 % 2](h1b, p1b)

                p2a = ps2.tile([128, c0], f32, tag="p2")
                if c1:
                    p2b = psA.tile([128, 64], f32, tag="p2b")
                for j in range(n_h1):
                    nc.tensor.matmul(
                        p2a, w2s[:, ts(j, 128)], h1a[:, j, :],
                        start=(j == 0), stop=(j == n_h1 - 1),
                    )
                    if c1:
                        nc.tensor.matmul(
                            p2b[:, :c1], w2s[:, ts(j, 128)], h1b[:, j * c1 : (j + 1) * c1],
                            start=(j == 0), stop=(j == n_h1 - 1),
                        )
                h2a = apool.tile([128, c0], mm_dt, tag="h2a")
                half = c0 // 2
                nc.scalar.activation(
                    h2a[:, :half], p2a[:, :half], mybir.ActivationFunctionType.Relu
                )
                nc.vector.tensor_scalar_max(h2a[:, half:], p2a[:, half:], 0.0)
                if c1:
                    h2b = apool.tile([128, c1], mm_dt, tag="h2b")
                    nc.vector.tensor_scalar_max(h2b, p2b[:, :c1], 0.0)

                p3a = ps3.tile([2 * NUM_ACTIONS, c0], f32, tag="p3")
                nc.tensor.matmul(p3a, whs, h2a, start=True, stop=True)
                if c1:
                    p3b = psB.tile([2 * NUM_ACTIONS, 64], f32, tag="p3b")
                    nc.tensor.matmul(p3b[:, :c1], whs, h2b, start=True, stop=True)

                ot = opool.tile([2 * NUM_ACTIONS, c0 + c1], OUT_DT, tag="ot")
                ha = c0 * 9 // 16 // 32 * 32
                nc.scalar.activation(
                    ot[:, :ha], p3a[:, :ha], mybir.ActivationFunctionType.Copy
                )
                nc.vector.tensor_copy(ot[:, ha:c0], p3a[:, ha:])
                if c1:
                    nc.vector.tensor_copy(ot[:, c0:], p3b[:, :c1])
                nc.sync.dma_start(out=outT[:, g0 : g0 + c0 + c1], in_=ot)

    nc.compile()
    return nc


def _prepare(state, option, linear1, linear2, mean_w, log_std_w):
    state = np.asarray(state, dtype=np.float32)
    option = np.asarray(option).astype(np.int64)
    linear1 = np.asarray(linear1, dtype=np.float32)
    linear2 = np.asarray(linear2, dtype=np.float32)
    mean_w = np.asarray(mean_w, dtype=np.float32)
    log_std_w = np.asarray(log_std_w, dtype=np.float32)

    batch = state.shape[0]
    np_dt = mybir.dt.np(MM_DT)

    counts = np.bincount(option, minlength=NUM_OPTIONS)
    cap = max(128, int(-(-counts.max() // 32) * 32))

    key = (cap, MM_DT)
    if key not in _kernel_cache:
        _kernel_cache[key] = _build(cap, MM_DT)
    nc = _kernel_cache[key]

    idx_per_opt = [np.nonzero(option == c)[0] for c in range(NUM_OPTIONS)]

    in_maps = []
    for c in range(NUM_OPTIONS):
        idx = idx_per_opt[c]
        a = np.zeros((128, cap + 2 * STATE_HIDDEN + 2 * NUM_ACTIONS), dtype=np_dt)
        a[:, : len(idx)] = state[idx].T
        a[:, cap : cap + STATE_HIDDEN] = linear1[c]
        w2p = (
            linear2[c]
            .reshape(STATE_HIDDEN // 128, 128, HIDDEN)
            .transpose(1, 0, 2)
            .reshape(128, STATE_HIDDEN)
        )
        a[:, cap + STATE_HIDDEN : cap + 2 * STATE_HIDDEN] = w2p
        a[:, cap + 2 * STATE_HIDDEN : cap + 2 * STATE_HIDDEN + NUM_ACTIONS] = mean_w[c]
        a[:, cap + 2 * STATE_HIDDEN + NUM_ACTIONS :] = log_std_w[c]
        in_maps.append({"a": a})

    return nc, in_maps, idx_per_opt, batch


def _unpack(res, idx_per_opt, batch):
    mean = np.empty((batch, NUM_ACTIONS), dtype=np.float32)
    log_std = np.empty((batch, NUM_ACTIONS), dtype=np.float32)
    for c in range(NUM_OPTIONS):
        idx = idx_per_opt[c]
        o = np.asarray(res.results[c]["outT"], dtype=np.float32)
        mean[idx] = o[:NUM_ACTIONS, : len(idx)].T
        log_std[idx] = o[NUM_ACTIONS:, : len(idx)].T
    np.clip(log_std, LOG_STD_MIN, LOG_STD_MAX, out=log_std)
    return mean, log_std


def kernel(state, option, linear1, linear2, mean_w, log_std_w):
    nc, in_maps, idx_per_opt, batch = _prepare(
        state, option, linear1, linear2, mean_w, log_std_w
    )
    res = run_bass_kernel_spmd(nc, in_maps, list(range(NUM_OPTIONS)))
    return _unpack(res, idx_per_opt, batch)


def timed_run(np_inputs):
    nc, in_maps, idx_per_opt, batch = _prepare(**np_inputs)
    res = run_bass_kernel_spmd(
        nc, in_maps, list(range(NUM_OPTIONS)), trace=True,
        trace_cores=list(range(NUM_OPTIONS)),
    )
    return res.exec_time_ns


# revision 8
# speedup vs baseline: 1.0235x; 1.0235x over previous
import os

import numpy as np

import concourse.bacc as bacc
import concourse.bass as bass
import concourse.mybir as mybir
import concourse.tile as tile
from concourse.bass import ts
from concourse.bass_utils import run_bass_kernel_spmd

NUM_OPTIONS = 8
NUM_INPUTS = 128
STATE_HIDDEN = 1024
HIDDEN = 128
NUM_ACTIONS = 32
LOG_STD_MIN = -20.0
LOG_STD_MAX = 2.0

MM_DT = getattr(mybir.dt, os.environ.get("KERNEL_MM_DT", "float16"))
OUT_DT = mybir.dt.float16
WARMUP_MMS = int(os.environ.get("KERNEL_WARMUP", "12"))

_kernel_cache: dict = {}


def _groups(cap: int) -> list[tuple[int, int, int]]:
    out = []
    s = 0
    while s < cap:
        n = min(cap - s, 576)
        c0 = min(n, 512)
        out.append((s, c0, n - c0))
        s += n
    return out


def _build(cap: int, mm_dt) -> bass.Bass:
    f32 = mybir.dt.float32
    n_h1 = STATE_HIDDEN // 128
    nc = bacc.Bacc(trn_type="TRN2", debug=False)

    awid = cap + 2 * STATE_HIDDEN + 2 * NUM_ACTIONS
    a = nc.dram_tensor("a", [128, awid], mm_dt, kind="ExternalInput").ap()
    outT = nc.dram_tensor("outT", [2 * NUM_ACTIONS, cap], OUT_DT, kind="ExternalOutput").ap()

    with tile.TileContext(nc) as tc:
        with (
            tc.tile_pool(name="ins", bufs=1) as ipool,
            tc.tile_pool(name="acts", bufs=1) as apool,
            tc.tile_pool(name="outs", bufs=1) as opool,
            tc.tile_pool(name="ps1", bufs=4, space="PSUM") as ps1,
            tc.tile_pool(name="psb", bufs=1, space="PSUM") as psb,
            tc.tile_pool(name="ps2", bufs=1, space="PSUM") as ps2,
            tc.tile_pool(name="ps3", bufs=1, space="PSUM") as ps3,
            tc.tile_pool(name="pse", bufs=1, space="PSUM") as pse,
        ):
            s1 = min(cap + 256, awid)
            s2 = min(s1 + 512, awid)
            s3 = min(s2 + 512, awid)
            asb = ipool.tile([128, awid], mm_dt)
            nc.sync.dma_start(out=asb[:, :s1], in_=a[:, :s1])
            if s1 < awid:
                nc.sync.dma_start(out=asb[:, s1:s2], in_=a[:, s1:s2])
            if s2 < awid:
                nc.sync.dma_start(out=asb[:, s2:s3], in_=a[:, s2:s3])
            if s3 < awid:
                nc.sync.dma_start(out=asb[:, s3:], in_=a[:, s3:])

            bf16 = mybir.dt.bfloat16
            wz = ipool.tile([128, 256], bf16)
            nc.gpsimd.memset(wz, 0)
            pw = ps2.tile([128, 256], f32, tag="p2")
            for _ in range(WARMUP_MMS):
                nc.tensor.matmul(pw, wz[:, :128], wz, start=True, stop=True)
            for _ in range(4):
                nc.tensor.matmul(
                    pw[:, :64], wz[:, :128], wz[:, :64], start=True, stop=True
                )

            xs = asb[:, :cap]
            w1s = asb[:, cap : cap + STATE_HIDDEN]
            w2s = asb[:, cap + STATE_HIDDEN : cap + 2 * STATE_HIDDEN]
            whs = asb[:, cap + 2 * STATE_HIDDEN :]

            relu_engines = (
                lambda o, i: nc.scalar.activation(o, i, mybir.ActivationFunctionType.Relu),
                lambda o, i: nc.vector.tensor_scalar_max(o, i, 0.0),
            )

            for g0, c0, c1 in _groups(cap):
                x0 = xs[:, g0 : g0 + c0]
                x1 = xs[:, g0 + c0 : g0 + c0 + c1]
                h1a = apool.tile([128, n_h1, c0], mm_dt, tag="h1a")
                if c1:
                    h1b = apool.tile([128, n_h1 * c1], mm_dt, tag="h1b")
                    p1b = psb.tile([128, n_h1 * c1], f32, tag="p1b")
                for j in range(n_h1):
                    p1 = ps1.tile([128, c0], f32, tag="p1")
                    nc.tensor.matmul(p1, w1s[:, ts(j, 128)], x0, start=True, stop=True)
                    if c1:
                        nc.tensor.matmul(
                            p1b[:, j * c1 : (j + 1) * c1], w1s[:, ts(j, 128)], x1,
                            start=True, stop=True,
                        )
                    relu_engines[j % 2](h1a[:, j, :], p1)
                if c1:
                    relu_engines[n_h1 % 2](h1b, p1b)

                p2a = ps2.tile([128, c0], f32, tag="p2")
                if c1:
                    pe_t = pse.tile([128, 128], f32, tag="pe")
                    p2b = pe_t[:, 0:64]
                for j in range(n_h1):
                    nc.tensor.matmul(
                        p2a, w2s[:, ts(j, 128)], h1a[:, j, :],
                        start=(j == 0), stop=(j == n_h1 - 1),
                    )
                    if c1:
                        nc.tensor.matmul(
                            p2b[:, :c1], w2s[:, ts(j, 128)], h1b[:, j * c1 : (j + 1) * c1],
                            start=(j == 0), stop=(j == n_h1 - 1),
                        )
                h2a = apool.tile([128, c0], mm_dt, tag="h2a")
                half = c0 // 2
                nc.scalar.activation(
                    h2a[:, :half], p2a[:, :half], mybir.ActivationFunctionType.Relu
                )
                nc.vector.tensor_scalar_max(h2a[:, half:], p2a[:, half:], 0.0)
                if c1:
                    h2b = apool.tile([128, c1], mm_dt, tag="h2b")
                    nc.vector.tensor_scalar_max(h2b, p2b[:, :c1], 0.0)

                p3a = ps3.tile([2 * NUM_ACTIONS, c0], f32, tag="p3")
                nc.tensor.matmul(p3a, whs, h2a, start=True, stop=True)
                ot = opool.tile([2 * NUM_ACTIONS, c0 + c1], OUT_DT, tag="ot")
                ha = c0 * 9 // 16 // 32 * 32
                nc.scalar.activation(
                    ot[:, :ha], p3a[:, :ha], mybir.ActivationFunctionType.Copy
                )
                nc.vector.tensor_copy(ot[:, ha:c0], p3a[:, ha:])
                if c1:
                    p3b = pe_t[0 : 2 * NUM_ACTIONS, 64:128]
                    nc.tensor.matmul(p3b[:, :c1], whs, h2b, start=True, stop=True)
                    nc.vector.tensor_copy(ot[:, c0:], p3b[:, :c1])
                nc.sync.dma_start(out=outT[:, g0 : g0 + c0 + c1], in_=ot)

    nc.compile()
    return nc


def _prepare(state, option, linear1, linear2, mean_w, log_std_w):
    state = np.asarray(state, dtype=np.float32)
    option = np.asarray(option).astype(np.int64)
    linear1 = np.asarray(linear1, dtype=np.float32)
    linear2 = np.asarray(linear2, dtype=np.float32)
    mean_w = np.asarray(mean_w, dtype=np.float32)
    log_std_w = np.asarray(log_std_w, dtype=np.float32)

    batch = state.shape[0]
    np_dt = mybir.dt.np(MM_DT)

    counts = np.bincount(option, minlength=NUM_OPTIONS)
    cap = max(128, int(-(-counts.max() // 32) * 32))

    key = (cap, MM_DT)
    if key not in _kernel_cache:
        _kernel_cache[key] = _build(cap, MM_DT)
    nc = _kernel_cache[key]

    idx_per_opt = [np.nonzero(option == c)[0] for c in range(NUM_OPTIONS)]

    in_maps = []
    for c in range(NUM_OPTIONS):
        idx = idx_per_opt[c]
        a = np.zeros((128, cap + 2 * STATE_HIDDEN + 2 * NUM_ACTIONS), dtype=np_dt)
        a[:, : len(idx)] = state[idx].T
        a[:, cap : cap + STATE_HIDDEN] = linear1[c]
        w2p = (
            linear2[c]
            .reshape(STATE_HIDDEN // 128, 128, HIDDEN)
            .transpose(1, 0, 2)
            .reshape(128, STATE_HIDDEN)
        )
        a[:, cap + STATE_HIDDEN : cap + 2 * STATE_HIDDEN] = w2p
        a[:, cap + 2 * STATE_HIDDEN : cap + 2 * STATE_HIDDEN + NUM_ACTIONS] = mean_w[c]
        a[:, cap + 2 * STATE_HIDDEN + NUM_ACTIONS :] = log_std_w[c]
        in_maps.append({"a": a})

    return nc, in_maps, idx_per_opt, batch


def _unpack(res, idx_per_opt, batch):
    mean = np.empty((batch, NUM_ACTIONS), dtype=np.float32)
    log_std = np.empty((batch, NUM_ACTIONS), dtype=np.float32)
    for c in range(NUM_OPTIONS):
        idx = idx_per_opt[c]
        o = np.asarray(res.results[c]["outT"], dtype=np.float32)
        mean[idx] = o[:NUM_ACTIONS, : len(idx)].T
        log_std[idx] = o[NUM_ACTIONS:, : len(idx)].T
    np.clip(log_std, LOG_STD_MIN, LOG_STD_MAX, out=log_std)
    return mean, log_std


def kernel(state, option, linear1, linear2, mean_w, log_std_w):
    nc, in_maps, idx_per_opt, batch = _prepare(
        state, option, linear1, linear2, mean_w, log_std_w
    )
    res = run_bass_kernel_spmd(nc, in_maps, list(range(NUM_OPTIONS)))
    return _unpack(res, idx_per_opt, batch)


def timed_run(np_inputs):
    nc, in_maps, idx_per_opt, batch = _prepare(**np_inputs)
    res = run_bass_kernel_spmd(
        nc, in_maps, list(range(NUM_OPTIONS)), trace=True,
        trace_cores=list(range(NUM_OPTIONS)),
    )
    return res.exec_time_ns
